# revision 32
# baseline (speedup 1.0000x reference)
"""ErrorAwareEdgeLoss Trainium2 kernel (batched-trace version).

Math: loss = mean_b [ (sum_e w_be * P[b,i_e,:] @ D @ P[b,j_e,:]) / max(sum_e w_be, 1e-8) ]

Reformulation:
    sum_e w_e * (P D P^T)[i_e, j_e] = <W_b, P_b D P_b^T>_F = <P_b^T W_b P_b, D>_F
where W_b[n,m] = sum_e w_e 1{i_e=n} 1{j_e=m} is the dense adjacency-weight
matrix (host-built from the edge list; the per-sample 1/max(sum w, eps)
normalization is folded into W). Summing over the batch BEFORE the dot:

    loss * B = < sum_b P_b^T W'_b P_b , D >_F,   W' = W / max(sum w, eps)

so the device runs, per batch, just 4 fp8 DoubleRow matmuls (T = W'' P,
then S += P^T T accumulated across all batches inside PSUM; W'' carries a
x1024 scale to stay clear of fp8 subnormals, undone in the host epilogue)
and finishes with a single <S, D> Frobenius dot on the DVE. No per-edge
traffic, no per-batch vector work, no DRAM spill. The T/S stages are
software-pipelined (T_b issues before S_{b-1}) so the PSUM->SBUF copy
round-trip never stalls the in-order PE queue, and a short warmup matmul
burst holds the PE p-state at full clock.

Sharding: data-parallel over batch: 8 NeuronCores x 8 batches. Each core
ships its [128, 2] partial dot; the host folds partitions and cores and
divides by B (the all-reduce of the sharding hint).
"""

from contextlib import ExitStack

import ml_dtypes
import numpy as np

import concourse.bacc as bacc
import concourse.bass as bass
import concourse.mybir as mybir
import concourse.tile as tile
from concourse.bass_utils import run_bass_kernel_spmd

B, N, E = 64, 256, 8192
NCORES = 8
BPC = B // NCORES  # batches per core

f32 = mybir.dt.float32
bf16 = mybir.dt.bfloat16
fp8 = mybir.dt.float8e4

NP_BF16 = ml_dtypes.bfloat16
NP_FP8 = ml_dtypes.float8_e4m3
WSCALE = 1024.0  # keeps W'/T values out of the fp8 subnormal range


def _build_bass():
    nc = bacc.Bacc("TRN2", target_bir_lowering=False, debug=False)

    pw_in = nc.dram_tensor("pw", [128, BPC, 4, N], fp8, kind="ExternalInput")
    d_in = nc.dram_tensor("derr", [128, 2, N], bf16, kind="ExternalInput")
    out = nc.dram_tensor("out", [128, 2, N], bf16, kind="ExternalOutput")

    with tile.TileContext(nc) as tc, ExitStack() as ctx:
        const_pool = ctx.enter_context(tc.tile_pool(name="const", bufs=1))
        in_pool = ctx.enter_context(tc.tile_pool(name="in", bufs=1))
        t_pool = ctx.enter_context(tc.tile_pool(name="t", bufs=2))
        psum_pool = ctx.enter_context(tc.tile_pool(name="ps", bufs=2, space="PSUM"))

        # ---- warm the PE p-state on a memset tile (no DMA dependency)
        warm_sb = const_pool.tile([128, N], bf16)
        nc.vector.memset(warm_sb[:], 0.0)
        warm_ps = psum_pool.tile([128, N], f32, tag="warm")
        for _ in range(10):
            nc.tensor.matmul(
                warm_ps[:], lhsT=warm_sb[:, :128], rhs=warm_sb[:],
                start=True, stop=True,
            )

        # ---- inputs: per-batch paired loads; arrival rate matches the PE's
        # ~1us/batch consumption so the pipeline never stalls (a monolithic
        # bulk load made batch 1 wait for the whole stream)
        pw_all = []
        for b in range(BPC):
            pw_sb = in_pool.tile([128, 4, N], fp8, tag=f"pw{b}")
            nc.sync.dma_start(pw_sb[:], pw_in[:, b])
            pw_all.append(pw_sb)
        d_sb = const_pool.tile([128, 2, N], bf16)
        nc.sync.dma_start(d_sb[:], d_in[:])

        # S[a, c] = sum_b sum_i P_b[i, a] T_b[i, c]; lives in PSUM all kernel
        s_ps0 = psum_pool.tile([128, N], f32, tag="s0")
        s_ps1 = psum_pool.tile([128, N], f32, tag="s1")
        s_ps = [s_ps0, s_ps1]

        # Software-pipelined: emit T_b, then S_{b-1}, so the PE computes
        # T_{b} while the Act engine copies T_{b-1} out of PSUM (the copy
        # round-trip would otherwise stall the in-order PE queue each batch).
        t_all = [None] * BPC
        for b in range(BPC + 1):
            if b < BPC:
                p_sb = pw_all[b][:, 0:2]
                wt_sb = pw_all[b][:, 2:4]
                # ---- T = W'' P : one fp8 DoubleRow matmul per out-chunk
                # (both j-tiles contracted in a single instruction)
                t_sb = t_pool.tile([128, 2, N], fp8)
                for ic in range(2):
                    t_ps = psum_pool.tile([128, N], f32, tag="tps")
                    nc.tensor.matmul(
                        t_ps[:],
                        lhsT=wt_sb[:, 0:2, ic * 128 : (ic + 1) * 128],
                        rhs=p_sb[:, 0:2, :],
                        start=True,
                        stop=True,
                        perf_mode=mybir.MatmulPerfMode.DoubleRow,
                    )
                    if b % 2 == 0:
                        nc.scalar.copy(t_sb[:, ic, :], t_ps[:])
                    else:
                        nc.vector.tensor_copy(t_sb[:, ic, :], t_ps[:])
                t_all[b] = t_sb
            if b >= 1:
                bp = b - 1
                p_prev = pw_all[bp][:, 0:2]
                t_prev = t_all[bp]
                # ---- S += P^T T (fp8 DoubleRow; PSUM accum across batches)
                for ac in range(2):
                    nc.tensor.matmul(
                        s_ps[ac][:],
                        lhsT=p_prev[:, 0:2, ac * 128 : (ac + 1) * 128],
                        rhs=t_prev[:, 0:2, :],
                        start=(bp == 0),
                        stop=(bp == BPC - 1),
                        perf_mode=mybir.MatmulPerfMode.DoubleRow,
                    )

        # ---- final: elementwise S*D off PSUM; the host folds the sum (the
        # reduce would only delay the last DMA, which gates the end barrier)
        prod = const_pool.tile([128, 2, N], bf16)
        for ac in range(2):
            nc.vector.tensor_tensor(
                out=prod[:, ac, :], in0=s_ps[ac][:], in1=d_sb[:, ac, :],
                op=mybir.AluOpType.mult,
            )
        nc.sync.dma_start(out[:], prod[:])

    if not nc.is_finalized():
        nc.finalize()
    return nc


_NC_CACHE = {}


def _get_nc():
    if "nc" not in _NC_CACHE:
        _NC_CACHE["nc"] = _build_bass()
    return _NC_CACHE["nc"]


def _prep_in_maps(P, d_error, edge_i, edge_j, edge_w):
    P = np.asarray(P, dtype=np.float32)
    d_error = np.asarray(d_error, dtype=np.float32)
    edge_i = np.asarray(edge_i, dtype=np.int32)
    edge_j = np.asarray(edge_j, dtype=np.int32)
    edge_w = np.asarray(edge_w, dtype=np.float32)

    # edge list -> dense, per-sample-normalized adjacency-weight matrix
    W = np.zeros((B, N * N), dtype=np.float32)
    flat = edge_i.astype(np.int64) * N + edge_j.astype(np.int64)
    np.add.at(W, (np.arange(B)[:, None], flat), edge_w)
    denom = np.maximum(edge_w.sum(axis=1), 1e-8)
    W = (W * (WSCALE / denom[:, None])).reshape(B, N, N)

    # P row-partitioned: p[b, p_, jc, c] = P[b, jc*128+p_, c]
    PL = P.reshape(B, 2, 128, N).transpose(0, 2, 1, 3).astype(NP_FP8)
    # W''^T row-partitioned: wt[b, p_, jc, i] = W''[b, i, jc*128+p_]
    WT = W.transpose(0, 2, 1).reshape(B, 2, 128, N).transpose(0, 2, 1, 3)
    WT = WT.astype(NP_FP8)
    # pack [P | W'^T] along the chunk axis -> one DMA per batch
    PW = np.concatenate([PL, WT], axis=2)  # [B, 128, 4, N]
    PW = PW.reshape(NCORES, BPC, 128, 4, N).transpose(0, 2, 1, 3, 4)
    # D row-partitioned: d[p_, ac, c] = D[ac*128+p_, c]
    D = np.ascontiguousarray(
        d_error.reshape(2, 128, N).transpose(1, 0, 2)
    ).astype(NP_BF16)

    in_maps = []
    for c in range(NCORES):
        in_maps.append(
            {
                "pw": np.ascontiguousarray(PW[c]),
                "derr": D,
            }
        )
    return in_maps


def run(P, d_error, edge_i, edge_j, edge_w, trace=False):
    """Run on 8 cores; returns (loss_scalar, BassKernelResults)."""
    nc = _get_nc()
    in_maps = _prep_in_maps(P, d_error, edge_i, edge_j, edge_w)
    res = run_bass_kernel_spmd(
        nc, in_maps, core_ids=list(range(NCORES)), trace=trace
    )
    total = np.float64(0.0)
    for r in res.results:
        total += np.asarray(r["out"], dtype=np.float64).sum()
    loss = np.float32(total / (B * np.float64(WSCALE)))
    return loss, res


def kernel(P, d_error, edge_i, edge_j, edge_w):
    loss, _ = run(P, d_error, edge_i, edge_j, edge_w, trace=False)
    return np.asarray(loss, dtype=np.float32)
